# revision 1
# baseline (speedup 1.0000x reference)
"""EvolveGCN kernel for 8 Trainium2 NeuronCores (Bass/Tile), v3.

Sharding: nodes 12500/core (padded 12544), edges partitioned by dst owner,
GRU weights row-sharded gate-aligned (tensor parallel), conv weights
replicated via a tiny AllGather of the GRU output.

v3 changes vs baseline:
  - fp16 everywhere on the edge path: table rows stored as 256B [x|x] fp16
    (dma_gather needs 256B granules), S one-hot fp16, reduce matmuls fp16
    (1 cycle/row vs 4 for fp32), GRU weights fp16.
  - One PSUM tile accumulates a whole dst window across all 4 table quarters
    (stream ordered window-major), closed by a single ACT copy into a
    persistent [65 x 12544] fp16 aggT; no SBUF adds, no memset.
  - Finalize fused: matmul K=65 (row 64 = sqrt(deg_in), rhs row 64 = bias)
    then one ACT op: Relu(scale*psum) with per-partition scale
    rs_in*rs_out (layer 1) / rs_in (layer 2).
  - S one-hot built 8 tiles per DVE instruction via stride-0 broadcast AP.
  - Gather calls of 2048 indices; call-padding indices are 0 but padded
    tiles emit no compute.
"""

import hashlib
import sys

import numpy as np

sys.path.insert(0, "/opt/trn_rl_repo")

N_NODES = 100000
D = 64
H = D * D                      # 4096
CORES = 8
SH = N_NODES // CORES          # 12500
SHP = 12544                    # padded shard (98*128)
NT = SHP // 128                # 98 node tiles
WN = 512                       # reduce window width (nodes)
NWIN = (SHP + WN - 1) // WN    # 25 windows (last is 256 wide)
NP = SHP * CORES               # 100352 table rows
CH = [3072, 3072, 3072, 3328]  # row-chunks of a shard (window-aligned: 6,6,6,7)
OFF = [0, 3072, 6144, 9216]
QG = [8 * c for c in CH]       # per-chunk table rows (int16-safe)
GW = [5, 11, 17, 24]           # last window of each chunk
GSL = H // CORES               # 512
CALL = 1024                    # gather idxs per call
TPC = CALL // 128              # 8 tiles per call
SB = 8                         # S-build batch (tiles per DVE op)

_cache = {}


def _host_prep(src, dst):
    """Index-side preprocessing: shard, sort, pad to a core-uniform layout."""
    src = np.asarray(src).astype(np.int64)
    dst = np.asarray(dst).astype(np.int64)
    deg_out = np.bincount(src, minlength=N_NODES).clip(min=1).astype(np.float32)
    deg_in = np.bincount(dst, minlength=N_NODES).clip(min=1).astype(np.float32)

    src_core = src // SH
    src_row = src % SH
    grp_of = np.searchsorted(np.array(OFF), src_row, side="right") - 1
    srel_of = src_core * np.array(CH)[grp_of] + (src_row - np.array(OFF)[grp_of])
    owner = dst // SH
    dst_rel = dst - owner * SH

    # bucket per (core, group, window)
    buckets = [[[None] * NWIN for _ in range(4)] for _ in range(CORES)]
    for c in range(CORES):
        m = owner == c
        s_g = grp_of[m]
        s_r = srel_of[m]
        dr = dst_rel[m]
        for g in range(4):
            gm = s_g == g
            gs, gd = s_r[gm], dr[gm]
            w = gd // WN
            for wi in range(NWIN):
                wm = w == wi
                buckets[c][g][wi] = (gs[wm], gd[wm])

    # uniform tile counts over cores
    T = np.zeros((4, NWIN), np.int64)
    for g in range(4):
        for wi in range(NWIN):
            mx = max(buckets[c][g][wi][0].size for c in range(CORES))
            T[g, wi] = -(-mx // 128) if mx else 0
    TG = [int(T[g].sum()) for g in range(4)]
    ncalls = [-(-t // TPC) for t in TG]

    # stream: window-major over (wi, g, k); per-group gather order follows it
    inst = []          # (g, t_in_g, col, wi, start, stop)
    col = 0
    t_in_g = [0, 0, 0, 0]
    for wi in range(NWIN):
        wtiles = []
        for g in range(4):
            for k in range(int(T[g, wi])):
                wtiles.append((g, t_in_g[g]))
                t_in_g[g] += 1
        for j, (g, t) in enumerate(wtiles):
            inst.append((g, t, col, wi, j == 0, j == len(wtiles) - 1))
            col += 1
    total_cols = col

    # per-core arrays: gather idx per group (stream order), dstw compare cols
    cores = []
    for c in range(CORES):
        idxs = [[] for _ in range(4)]
        cmps = np.full((total_cols, 128), -1.0, np.float16)
        ci = 0
        for wi in range(NWIN):
            for g in range(4):
                gs, gd = buckets[c][g][wi]
                n = gs.size
                tot = int(T[g, wi]) * 128
                if tot:
                    idx = np.zeros(tot, np.int64)
                    idx[:n] = gs
                    idxs[g].append(idx)
                    cm = np.full(tot, -1.0, np.float32)
                    cm[:n] = gd - wi * WN
                    cmps[ci:ci + tot // 128] = \
                        cm.reshape(-1, 128).astype(np.float16)
                    ci += tot // 128
        assert ci == total_cols
        idx16 = []
        for g in range(4):
            v = np.concatenate(idxs[g]) if idxs[g] else np.zeros(0, np.int64)
            v = np.concatenate([v, np.zeros(ncalls[g] * CALL - v.size,
                                            np.int64)])
            v = v.astype(np.int16).reshape(-1, 16).T    # [16, n/16]
            idx16.append(np.tile(v, (8, 1)).copy())
        dstw = np.ascontiguousarray(cmps.T)             # [128, total_cols]
        cores.append(dict(idx16=idx16, dstw=dstw))

    struct = dict(T=T, TG=TG, ncalls=ncalls, inst=inst,
                  total_cols=total_cols)
    return cores, struct, deg_out, deg_in


def _pad_shard(a, c, fill=0.0):
    sh = a[c * SH:(c + 1) * SH]
    pad = np.full((SHP - SH,) + a.shape[1:], fill, a.dtype)
    return np.concatenate([sh, pad], axis=0)


def _build(struct):
    from concourse import bacc, bass, mybir
    import concourse.tile as tile
    import contextlib

    f32 = mybir.dt.float32
    f16 = mybir.dt.float16
    i16 = mybir.dt.int16
    ncalls = struct["ncalls"]
    inst = struct["inst"]
    total_cols = struct["total_cols"]

    nc = bacc.Bacc("TRN2", target_bir_lowering=False, debug=False,
                   num_devices=CORES)

    xsh = nc.dram_tensor("xsh", [SHP, D], f32, kind="ExternalInput")
    dego = nc.dram_tensor("dego", [128, NT], f32, kind="ExternalInput")
    degi = nc.dram_tensor("degi", [128, NT], f32, kind="ExternalInput")
    dsqw = nc.dram_tensor("dsqw", [1, SHP], f16, kind="ExternalInput")
    wihT = nc.dram_tensor("wihT", [H, 3 * GSL], f16, kind="ExternalInput")
    whhT = nc.dram_tensor("whhT", [H, 3 * GSL], f16, kind="ExternalInput")
    xrhs = nc.dram_tensor("xrhs", [H, 2], f16, kind="ExternalInput")
    hrhs = nc.dram_tensor("hrhs", [H, 2], f16, kind="ExternalInput")
    bih = nc.dram_tensor("bih", [2, 3 * GSL], f32, kind="ExternalInput")
    bhh = nc.dram_tensor("bhh", [2, 3 * GSL], f32, kind="ExternalInput")
    hsl = nc.dram_tensor("hsl", [2, GSL], f32, kind="ExternalInput")
    b1h = nc.dram_tensor("b1h", [1, D], f16, kind="ExternalInput")
    b2h = nc.dram_tensor("b2h", [1, D], f16, kind="ExternalInput")
    iotain = nc.dram_tensor("iotain", [128, SB * WN], f16,
                            kind="ExternalInput")
    idx_in = [nc.dram_tensor(f"idx{g}", [128, ncalls[g] * CALL // 16], i16,
                             kind="ExternalInput") for g in range(4)]
    dstw_in = nc.dram_tensor("dstw", [128, total_cols], f16,
                             kind="ExternalInput")
    y = nc.dram_tensor("y", [SHP, D], f32, kind="ExternalOutput")

    xb1 = nc.dram_tensor("xb1", [SHP, D], f32, kind="Internal")
    xb2 = nc.dram_tensor("xb2", [SHP, D], f32, kind="Internal")
    tabq1 = [nc.dram_tensor(f"tabq1_{g}", [QG[g], D], f32, kind="Internal",
                            addr_space="Shared") for g in range(4)]
    tabq2 = [nc.dram_tensor(f"tabq2_{g}", [QG[g], D], f32, kind="Internal",
                            addr_space="Shared") for g in range(4)]
    wnew = nc.dram_tensor("wnew", [2, GSL], f32, kind="Internal")
    wg = nc.dram_tensor("wg", [2 * CORES, GSL], f32, kind="Internal",
                        addr_space="Shared")

    with tile.TileContext(nc) as tc:
        with contextlib.ExitStack() as ctx:
            sp = ctx.enter_context(tc.tile_pool(name="persist", bufs=1))
            xp = ctx.enter_context(tc.tile_pool(name="xtiles", bufs=4))
            gp = ctx.enter_context(tc.tile_pool(name="gather", bufs=8))
            spl = ctx.enter_context(tc.tile_pool(name="sbuf_s", bufs=3))
            grup = ctx.enter_context(tc.tile_pool(name="gru", bufs=4))
            finp = ctx.enter_context(tc.tile_pool(name="fin", bufs=4))
            ps_red = ctx.enter_context(
                tc.tile_pool(name="psred", bufs=2, space="PSUM"))
            ps_gru = ctx.enter_context(
                tc.tile_pool(name="psgru", bufs=2, space="PSUM"))
            ps_fin = ctx.enter_context(
                tc.tile_pool(name="psfin", bufs=2, space="PSUM"))

            iota8 = sp.tile([128, SB * WN], f16)
            nc.sync.dma_start(iota8[:], iotain.ap())
            rs_i = sp.tile([128, NT], f32, tag="rs_i")
            rs_o = sp.tile([128, NT], f32, tag="rs_o")
            rs_io = sp.tile([128, NT], f32, tag="rs_io")
            dl1 = sp.tile([128, NT], f32, tag="dl1")
            nc.sync.dma_start(dl1[:], degi.ap())
            nc.vector.reciprocal(dl1[:], dl1[:])
            nc.scalar.activation(rs_i[:], dl1[:],
                                 mybir.ActivationFunctionType.Sqrt)
            dl2 = sp.tile([128, NT], f32, tag="dl2")
            nc.sync.dma_start(dl2[:], dego.ap())
            nc.vector.reciprocal(dl2[:], dl2[:])
            nc.scalar.activation(rs_o[:], dl2[:],
                                 mybir.ActivationFunctionType.Sqrt)
            nc.vector.tensor_mul(rs_io[:], rs_i[:], rs_o[:])

            # aggT: rows 0-63 window accumulators, row 64 = sqrt(deg_in)
            aggT = sp.tile([65, SHP], f16)
            nc.sync.dma_start(aggT[64:65, :], dsqw.ap())

            # scaled x -> xb1 (dup cols) -> AllGather tab1
            xv = xsh.ap().rearrange("(a p) d -> a p d", p=128)
            bv1 = xb1.ap().rearrange("(a p) d -> a p d", p=128)
            for a in range(NT):
                xt = xp.tile([128, D], f32, tag="xl")
                nc.sync.dma_start(xt[:], xv[a])
                xs = xp.tile([128, D], f32, tag="xs")
                nc.vector.tensor_scalar_mul(xs[:], xt[:], rs_o[:, a:a + 1])
                nc.sync.dma_start(bv1[a], xs[:])
            for g in range(4):
                nc.gpsimd.collective_compute(
                    "AllGather", mybir.AluOpType.bypass,
                    replica_groups=[list(range(CORES))],
                    ins=[xb1.ap()[OFF[g]:OFF[g] + CH[g], :]],
                    outs=[tabq1[g].ap()])

            # GRU (fp16 weights, fp32 accumulation)
            xck = []
            for k in range(H // 128):
                t = sp.tile([128, 2], f16, tag=f"xc{k}")
                nc.sync.dma_start(
                    t[:], xrhs.ap().rearrange("(k p) t -> k p t", p=128)[k])
                xck.append(t)
            hck = []
            for k in range(H // 128):
                t = sp.tile([128, 2], f16, tag=f"hc{k}")
                nc.sync.dma_start(
                    t[:], hrhs.ap().rearrange("(k p) t -> k p t", p=128)[k])
                hck.append(t)

            def gru_matvec(wT, lhs_list, out_sb):
                for j in range(3):
                    ps = ps_gru.tile([2, GSL], f32)
                    for k in range(H // 128):
                        rt = grup.tile([128, GSL], f16, tag="rt")
                        nc.sync.dma_start(
                            rt[:], wT.ap()[k * 128:(k + 1) * 128,
                                           j * GSL:(j + 1) * GSL])
                        nc.tensor.matmul(ps[:], lhs_list[k][:], rt[:],
                                         start=(k == 0),
                                         stop=(k == H // 128 - 1))
                    nc.vector.tensor_copy(out_sb[:, j * GSL:(j + 1) * GSL],
                                          ps[:])

            gx = sp.tile([2, 3 * GSL], f32, tag="gx")
            gh = sp.tile([2, 3 * GSL], f32, tag="gh")
            gru_matvec(wihT, xck, gx)
            gru_matvec(whhT, hck, gh)
            bt1 = sp.tile([2, 3 * GSL], f32, tag="bt1")
            nc.sync.dma_start(bt1[:], bih.ap())
            nc.vector.tensor_add(gx[:], gx[:], bt1[:])
            bt2 = sp.tile([2, 3 * GSL], f32, tag="bt2")
            nc.sync.dma_start(bt2[:], bhh.ap())
            nc.vector.tensor_add(gh[:], gh[:], bt2[:])
            S0 = slice(0, GSL)
            S1 = slice(GSL, 2 * GSL)
            S2 = slice(2 * GSL, 3 * GSL)
            r = sp.tile([2, GSL], f32, tag="r")
            nc.vector.tensor_add(r[:], gx[:, S0], gh[:, S0])
            nc.scalar.activation(r[:], r[:],
                                 mybir.ActivationFunctionType.Sigmoid)
            z = sp.tile([2, GSL], f32, tag="z")
            nc.vector.tensor_add(z[:], gx[:, S1], gh[:, S1])
            nc.scalar.activation(z[:], z[:],
                                 mybir.ActivationFunctionType.Sigmoid)
            n_ = sp.tile([2, GSL], f32, tag="n")
            nc.vector.tensor_mul(n_[:], r[:], gh[:, S2])
            nc.vector.tensor_add(n_[:], n_[:], gx[:, S2])
            nc.scalar.activation(n_[:], n_[:],
                                 mybir.ActivationFunctionType.Tanh)
            ht = sp.tile([2, GSL], f32, tag="ht")
            nc.sync.dma_start(ht[:], hsl.ap())
            wn_t = sp.tile([2, GSL], f32, tag="wn")
            nc.vector.tensor_sub(wn_t[:], ht[:], n_[:])
            nc.vector.tensor_mul(wn_t[:], z[:], wn_t[:])
            nc.vector.tensor_add(wn_t[:], n_[:], wn_t[:])
            nc.sync.dma_start(wnew.ap(), wn_t[:])
            # finalize rhs tiles (filled by emit_wg_tail inside layer 1)
            wb1 = sp.tile([65, 64], f16, tag="wb1")
            wb2 = sp.tile([65, 64], f16, tag="wb2")

            def emit_wg_tail():
                nc.gpsimd.collective_compute(
                    "AllGather", mybir.AluOpType.bypass,
                    replica_groups=[list(range(CORES))],
                    ins=[wnew.ap()], outs=[wg.ap()])
                w1t = sp.tile([64, 64], f32, tag="w1t")
                w2t = sp.tile([64, 64], f32, tag="w2t")
                for i in range(CORES):
                    nc.sync.dma_start(
                        w1t[8 * i:8 * i + 8, :],
                        wg.ap()[2 * i:2 * i + 1, :].rearrange(
                            "a (b d) -> (a b) d", d=64))
                    nc.sync.dma_start(
                        w2t[8 * i:8 * i + 8, :],
                        wg.ap()[2 * i + 1:2 * i + 2, :].rearrange(
                            "a (b d) -> (a b) d", d=64))
                nc.vector.tensor_copy(wb1[0:64, :], w1t[:])
                nc.sync.dma_start(wb1[64:65, :], b1h.ap())
                nc.vector.tensor_copy(wb2[0:64, :], w2t[:])
                nc.sync.dma_start(wb2[64:65, :], b2h.ap())

            idx_sb = []
            for g in range(4):
                it = sp.tile([128, ncalls[g] * CALL // 16], i16,
                             tag=f"idx{g}")
                nc.sync.dma_start(it[:], idx_in[g].ap())
                idx_sb.append(it)
            dstw_sb = sp.tile([128, total_cols], f16, tag="dstw")
            nc.sync.dma_start(dstw_sb[:], dstw_in.ap())

            FDELAY = 4

            def layer(tabs_, wb, relu, scale_ap_tile, out_bv, hooks):
                gts = {}

                def ensure_call(g, cb):
                    if (g, cb) in gts:
                        return
                    gt = gp.tile([128, TPC, D], f32, tag="gt")
                    nc.gpsimd.dma_gather(
                        out_ap=gt[:],
                        in_ap=tabs_[g].ap(),
                        idxs_ap=idx_sb[g][:, cb * (CALL // 16):
                                          (cb + 1) * (CALL // 16)],
                        num_idxs=CALL, num_idxs_reg=CALL, elem_size=D)
                    gt16 = gp.tile([128, TPC, D], f16, tag="gt16")
                    nc.scalar.activation(gt16[:], gt[:],
                                         mybir.ActivationFunctionType.Copy)
                    gts[(g, cb)] = gt16

                cur_s = [None, -1]     # tile, base col
                open_ps = [None]

                def fin_window(wi):
                    # finalize node tiles of window wi
                    c0 = (wi * WN) // 128
                    c1 = min(((wi + 1) * WN), SHP) // 128
                    for a in range(c0, c1):
                        ps2 = ps_fin.tile([128, 64], f32)
                        nc.tensor.matmul(
                            ps2[:], aggT[:, a * 128:(a + 1) * 128], wb[:],
                            start=True, stop=True)
                        ot = finp.tile([128, D], f32, tag="ot")
                        nc.scalar.activation(
                            ot[:], ps2[:],
                            mybir.ActivationFunctionType.Relu if relu
                            else mybir.ActivationFunctionType.Copy,
                            scale=scale_ap_tile[:, a:a + 1])
                        nc.sync.dma_start(out_bv[a], ot[:])

                for (g, t_in_g, col, wi, st, sp_) in inst:
                    ensure_call(g, t_in_g // TPC)
                    gt = gts[(g, t_in_g // TPC)]
                    sub = t_in_g % TPC
                    if col % SB == 0:
                        nb = min(SB, total_cols - col)
                        s_t = spl.tile([128, SB * WN], f16, tag="S")
                        nc.vector.tensor_tensor(
                            out=s_t[:, 0:nb * WN].rearrange(
                                "p (b w) -> p b w", b=nb),
                            in0=dstw_sb[:, col:col + nb].to_broadcast(
                                [128, nb, WN]),
                            in1=iota8[:, 0:nb * WN].rearrange(
                                "p (b w) -> p b w", b=nb),
                            op=mybir.AluOpType.is_equal)
                        cur_s = [s_t, col]
                    if st:
                        open_ps[0] = ps_red.tile([64, WN], f32, name="pw",
                                                 tag="pw")
                    b = col - cur_s[1]
                    nc.tensor.matmul(open_ps[0][:],
                                     gt[:, sub, :],
                                     cur_s[0][:, b * WN:(b + 1) * WN],
                                     start=st, stop=sp_)
                    if sp_:
                        w0 = wi * WN
                        w1 = min((wi + 1) * WN, SHP)
                        nc.scalar.activation(
                            aggT[0:64, w0:w1], open_ps[0][:, 0:w1 - w0],
                            mybir.ActivationFunctionType.Copy)
                        if wi >= FDELAY:
                            fin_window(wi - FDELAY)
                        if wi in hooks:
                            hooks[wi]()
                for wi in range(NWIN - FDELAY, NWIN):
                    fin_window(wi)

            bv2 = xb2.ap().rearrange("(a p) d -> a p d", p=128)

            def l2_ag(g):
                def h():
                    nc.gpsimd.collective_compute(
                        "AllGather", mybir.AluOpType.bypass,
                        replica_groups=[list(range(CORES))],
                        ins=[xb2.ap()[OFF[g]:OFF[g] + CH[g], :]],
                        outs=[tabq2[g].ap()])
                return h

            # L2 chunk g is complete once window GW[g]+FDELAY has finalized;
            # with the FDELAY lag that is at the close of window GW[g]+FDELAY.
            hooks1 = {2: emit_wg_tail}
            for g in range(4):
                wtrig = GW[g] + FDELAY
                if wtrig >= NWIN:
                    wtrig = None          # emit after the layer loop
                if wtrig is not None:
                    hooks1.setdefault(wtrig, lambda: None)
                    prev = hooks1[wtrig]
                    hooks1[wtrig] = (lambda p=prev, hh=l2_ag(g): (p(), hh()))
            layer(tabq1, wb1, relu=True, scale_ap_tile=rs_io, out_bv=bv2,
                  hooks=hooks1)
            l2_ag(3)()
            yv = y.ap().rearrange("(a p) d -> a p d", p=128)
            layer(tabq2, wb2, relu=False, scale_ap_tile=rs_i, out_bv=yv,
                  hooks={})

    nc.compile()
    return nc


def kernel(node_embeddings, src, dst, gc1_weight, gc1_bias, gc2_weight,
           gc2_bias, gc1_hist, gc2_hist, gru_w_ih, gru_w_hh, gru_b_ih,
           gru_b_hh):
    from concourse import bass_utils

    node_embeddings = np.asarray(node_embeddings, dtype=np.float32)
    src_i = np.asarray(src)
    dst_i = np.asarray(dst)
    cores, struct, deg_out, deg_in = _host_prep(src_i, dst_i)

    skey = hashlib.sha1(b"v5" + src_i.tobytes() + dst_i.tobytes()).hexdigest()
    if skey not in _cache:
        _cache[skey] = _build(struct)
    nc = _cache[skey]

    w1f = np.asarray(gc1_weight, np.float32).reshape(-1)
    w2f = np.asarray(gc2_weight, np.float32).reshape(-1)
    h1f = np.asarray(gc1_hist, np.float32).reshape(-1)
    h2f = np.asarray(gc2_hist, np.float32).reshape(-1)
    wih = np.asarray(gru_w_ih, np.float32)
    whh = np.asarray(gru_w_hh, np.float32)
    bihv = np.asarray(gru_b_ih, np.float32)
    bhhv = np.asarray(gru_b_hh, np.float32)
    iota8 = np.tile(np.arange(WN, dtype=np.float16), (128, SB))

    def lay_deg(d, c):
        p = _pad_shard(d.reshape(N_NODES, 1), c, fill=1.0).reshape(SHP)
        return p.reshape(NT, 128).T.copy()

    in_maps = []
    for c in range(CORES):
        rows = np.concatenate([np.arange(c * GSL, (c + 1) * GSL),
                               H + np.arange(c * GSL, (c + 1) * GSL),
                               2 * H + np.arange(c * GSL, (c + 1) * GSL)])
        dsq = np.sqrt(_pad_shard(deg_in.reshape(N_NODES, 1), c,
                                 fill=1.0).reshape(SHP))
        m = {
            "xsh": _pad_shard(node_embeddings, c),
            "dego": lay_deg(deg_out, c),
            "degi": lay_deg(deg_in, c),
            "dsqw": dsq.astype(np.float16).reshape(1, SHP),
            "wihT": np.ascontiguousarray(wih[rows, :].T).astype(np.float16),
            "whhT": np.ascontiguousarray(whh[rows, :].T).astype(np.float16),
            "xrhs": np.ascontiguousarray(
                np.stack([h1f, h2f], axis=1)).astype(np.float16),
            "hrhs": np.ascontiguousarray(
                np.stack([w1f, w2f], axis=1)).astype(np.float16),
            "bih": np.tile(bihv[rows], (2, 1)),
            "bhh": np.tile(bhhv[rows], (2, 1)),
            "hsl": np.ascontiguousarray(
                np.stack([w1f[c * GSL:(c + 1) * GSL],
                          w2f[c * GSL:(c + 1) * GSL]])),
            "b1h": np.asarray(gc1_bias, np.float16).reshape(1, D),
            "b2h": np.asarray(gc2_bias, np.float16).reshape(1, D),
            "iotain": iota8,
            "dstw": cores[c]["dstw"],
        }
        for g in range(4):
            m[f"idx{g}"] = cores[c]["idx16"][g]
        in_maps.append(m)

    import os
    trace = False
    if os.environ.get("KERNEL_TRACE") == "1":
        try:
            _install_ntff_hook()
            trace = True
        except Exception:
            trace = False
    res = bass_utils.run_bass_kernel_spmd(nc, in_maps,
                                          core_ids=list(range(CORES)),
                                          trace=trace)
    global last_exec_time_ns
    last_exec_time_ns = res.exec_time_ns
    out = np.concatenate([res.results[c]["y"][:SH] for c in range(CORES)],
                         axis=0)
    return out.astype(np.float32)


last_exec_time_ns = None


def _install_ntff_hook():
    """Register the NTFF profile hook trn_boot couldn't (missing
    antenv.axon_hooks in this image). Test-only; guarded by KERNEL_TRACE."""
    import types
    import antenv

    if "antenv.axon_hooks" in sys.modules:
        return
    holder = {"h": None}
    mod = types.ModuleType("antenv.axon_hooks")
    mod.get_axon_ntff_profile_hook = lambda: holder["h"]
    mod.set_axon_ntff_profile_hook = lambda h: holder.update(h=h)
    sys.modules["antenv.axon_hooks"] = mod
    antenv.axon_hooks = mod
    sys.path.insert(0, "/root/.axon_site")
    from trn_agent_boot.trn_boot import _ntff_profile_via_ctypes
    holder["h"] = _ntff_profile_via_ctypes("/opt/axon/libaxon_pjrt.so")

